# revision 39
# baseline (speedup 1.0000x reference)
"""KMeans assignment kernel for TRN2 (8 NeuronCores, data-parallel over points).

Computes argmin_k ||x_n - c_k||^2 for x (65536, 512) f32, centers (4096, 512) f32.

Strategy v6 (fp16 matmul + ONE custom fused bias+argmax DVE op per half):
  - argmin_k dist = argmax_k s,  s = 2*x.c_k - ||c_k||^2   (x-norm constant per row)
  - ONE matmul pass p = (2x) @ c^T in fp16 (measured ~222ns/512-col MM vs
    255ns fp32r - the fp16 gain is the 2x faster FWL weight load; the array
    streams 1 col/cycle for every dtype). fp16 operand rounding: 37/65536
    argmax flips measured on the actual data (rel err 1.60e-2, under the
    2e-2 gate). NO bias matmuls (each costs a full 512 cols = 20% PE).
  - Per half (4 banks), ONE custom DVE instruction (registered at import into
    concourse's per-NEFF custom-op table; no firmware change) reads PSUM and
    computes, in a single 1-elem/cycle pass:
        s    = Src0 + Src1              # p + bias (exact f32 bias, Src1 full tensor)
        r    = scan(MAX, s)             # running max
        out  = select(s == r, Idx, r)   # Idx at prefix-maxima (>=0), else r (<0)
        accum= MAX(out)                 # last prefix-max position = argmax_k s
    Scores are always negative (s <= -50 at 8.5 sigma) so Idx >= 0 dominates
    r in the accum, and out[:, -1] = r[-1] = the half's max value m_h -
    unless the argmax IS the last element (accum == 2047), in which case the
    host recomputes those ~2/2048 points exactly (tiny numpy matmul).
    (The stock tensor_tensor_reduce NRT-faults this HW build in every
    variant; max8/max_index/tensor_reduce+select need 2 full DVE passes.)
  - Act does only the two [128,1] copies of out[:, -1] into the m staging
    tile. Host picks the winning half per point (m1 > m0, ties -> half 0 =
    jnp's first-index tiebreak): idx = 2048*h + j_h.
  - Engine budget/tile: PE 32 MMs at the 215.8ns/MM decode floor (~6.9us,
    ~95% busy = the wall), DVE 2 fused passes ~4.8us, Act ~0.6us. Measured
    ~472us total (442 PE + ~12 prologue DMA + ~8 drain tail) vs 688us for
    the fp32r max8/max_index baseline.
  - Data-parallel: 8192 points/core, centers replicated; no collectives.
"""
import os
import numpy as np

import concourse.bass as bass
import concourse.bacc as bacc
import concourse.tile as tile
import concourse.mybir as mybir
from concourse.bass_utils import run_bass_kernel_spmd

N_CORES = 8
N_POINTS = 65536
K = 4096
F = 512
PTS_PER_CORE = N_POINTS // N_CORES      # 8192
NT = PTS_PER_CORE // 128                # 64 x-tiles per core
NFC = F // 128                          # 4 contraction chunks
NB = 4                                  # banks per PSUM half
KH = K // 2                             # 2048 centers per half
F32 = mybir.dt.float32
F16 = mybir.dt.float16

_NC = None
LAST_BR = None
_ARGMAX_OP = None


def _get_argmax_op():
    """Register (once) the fused bias-add + running-max + argmax custom DVE
    op in concourse's custom-op registry. The uop program is per-NEFF table
    data; shas are computed here so the pin always matches this build."""
    global _ARGMAX_OP
    if _ARGMAX_OP is not None:
        return _ARGMAX_OP
    import concourse.dve_ops as dve_ops_mod
    from concourse.dve_ops import DveOp, OPS
    from concourse.dve_spec import (
        Spec, Src0, Src1, AluOp, Idx, scan, eq, select, lower,
    )
    from concourse.dve_uop import DveOpSpec

    name = "ARGMAX_BIAS_ANT"
    for op in OPS:
        if op.name == name:
            _ARGMAX_OP = op
            return op

    s = Src0 + Src1
    r = scan(AluOp.MAX, s)
    body = select(eq(s, r), Idx, r)

    def ref(in0, in1, s0, s1, imm2):
        P = in0.shape[0]
        ss = (np.asarray(in0, np.float32).reshape(P, -1)
              + np.asarray(in1, np.float32).reshape(P, -1))
        rr = np.maximum.accumulate(ss, axis=1)
        idx = np.broadcast_to(
            np.arange(ss.shape[1], dtype=np.float32), ss.shape)
        out = np.where(ss == rr, idx, rr).astype(np.float32)
        return out.reshape(in0.shape), out.max(axis=1)

    spec = Spec(body=body, accum=AluOp.MAX, reference=ref)
    row = dve_ops_mod._CUSTOM_DVE_ROW_BASE + len(OPS)
    shas = {}
    for ver in ("v3", "v4"):
        op_spec = DveOpSpec(name=name, opcode=row,
                            uops=lower(spec, ver=ver), rd1_en=True)
        shas[ver] = op_spec.sha(ver)
    op = DveOp(name, spec, subdim=False, uops_sha=shas)
    OPS.append(op)
    dve_ops_mod.CUSTOM_DVE_SPECS[name] = spec
    dve_ops_mod._SUB_OPCODE_FOR_NAME[name] = row
    _ARGMAX_OP = op
    return op


def _build():
    argmax_op = _get_argmax_op()
    nc = bacc.Bacc("TRN2", target_bir_lowering=False, debug=False,
                   num_devices=N_CORES)
    xh_d = nc.declare_dram_parameter("xh", [NT, 128, NFC, 128], F16, isOutput=False)
    ch_d = nc.declare_dram_parameter("ch", [NFC, 2, 128, KH], F16, isOutput=False)
    bias_d = nc.declare_dram_parameter("biasr", [2, 128, KH], F32,
                                       isOutput=False)
    oa_d = nc.declare_dram_parameter("oacc", [128, NT, 2], F32, isOutput=True)
    om_d = nc.declare_dram_parameter("omax", [128, NT, 2], F32, isOutput=True)

    with tile.TileContext(nc) as tc:
        with (
            tc.tile_pool(name="const", bufs=1) as cpool,
            tc.tile_pool(name="xp", bufs=4) as xpool,
            tc.tile_pool(name="jk", bufs=3) as jkpool,
            tc.tile_pool(name="st", bufs=1) as stpool,
            tc.tile_pool(name="ps", bufs=1, space="PSUM") as pspool,
        ):
            # Prologue DMAs in first-consumption order. ch chunks stream on
            # the low-latency HWDGE scalar queue; the bias halves ride the
            # (mostly idle) sync queue in quarter chunks so they land before
            # tile 0's DVE ops without ever blocking the ch stream.
            # ch chunks on the low-latency HWDGE scalar queue (first chunk
            # lands ~2us earlier than SWDGE); bias rides the gpsimd queue.
            chs = [[None, None] for _ in range(NFC)]
            biast = cpool.tile([128, 2, KH], F32, tag="biasr")
            for h in range(2):
                for fc in range(NFC):
                    cht = cpool.tile([128, KH], F16, tag=f"ch{fc}_{h}",
                                     name=f"ch{fc}_{h}")
                    nc.scalar.dma_start(cht[:], ch_d[fc, h])
                    chs[fc][h] = cht
                nc.gpsimd.dma_start(biast[:, h], bias_d[h])

            ast = stpool.tile([128, NT, 2], F32, tag="ast")
            mst = stpool.tile([128, NT, 2], F32, tag="mst")

            for t in range(NT):
                xt = xpool.tile([128, NFC * 128], F16, tag="x")
                nc.sync.dma_start(xt[:], xh_d[t])

                for h in range(2):
                    ph = pspool.tile([128, NB, 512], F32, tag=f"p{h}",
                                     name=f"p{h}")
                    for fc in range(NFC):
                        for b in range(NB):
                            nc.tensor.matmul(
                                ph[:, b, :],
                                xt[:, fc * 128:(fc + 1) * 128],
                                chs[fc][h][:, b * 512:(b + 1) * 512],
                                start=(fc == 0),
                                stop=(fc == NFC - 1),
                            )
                    # ONE fused pass: bias-add + running max + argmax accum.
                    junk = jkpool.tile([128, KH], F32, tag="junk")
                    nc.vector._custom_dve(
                        argmax_op,
                        out=junk[:],
                        in0=ph.rearrange("p b f -> p (b f)"),
                        in1=biast[:, h],
                        accum_out=ast[:, t, h:h + 1],
                    )
                    # m_h = the running max's final value (valid unless the
                    # argmax is the last element - host repairs those).
                    nc.scalar.copy(mst[:, t, h:h + 1], junk[:, KH - 1:KH])

                # Chunked output DMA on the sync queue (near-zero drain cost
                # at block exit): drains all but the last 16 tiles' results
                # before the loop ends.
                if t % 16 == 15:
                    t0 = t - 15
                    nc.sync.dma_start(oa_d[:, t0:t + 1], ast[:, t0:t + 1])
                    nc.sync.dma_start(om_d[:, t0:t + 1], mst[:, t0:t + 1])
    nc.compile()
    return nc


def _get_nc():
    global _NC
    if _NC is None:
        _NC = _build()
    return _NC


def kernel(x: np.ndarray, centers: np.ndarray) -> np.ndarray:
    global LAST_BR, _LAST_IN_MAPS
    x = np.ascontiguousarray(x, dtype=np.float32)
    centers = np.ascontiguousarray(centers, dtype=np.float32)

    v16 = (2.0 * x).astype(np.float16)
    c16 = centers.astype(np.float16)

    # pack x side: [core, t, fp, fc, j] <- v[core*8192 + t*128 + j, fc*128 + fp]
    a = v16.reshape(N_CORES, NT, 128, NFC, 128)       # [core, t, j, fc, fp]
    xh_p = np.ascontiguousarray(a.transpose(0, 1, 4, 3, 2))

    # pack c side: [fc, h, fp, kh] <- c[h*2048 + kh, fc*128 + fp]
    c = c16.reshape(2, KH, NFC, 128)                  # [h, kh, fc, fp]
    ch_p = np.ascontiguousarray(c.transpose(2, 0, 3, 1))

    bias = (-(centers.astype(np.float64) ** 2).sum(axis=1)).astype(np.float32)
    bias_p = np.ascontiguousarray(
        np.broadcast_to(bias.reshape(2, 1, KH), (2, 128, KH)))

    in_maps = [
        {"xh": xh_p[i], "ch": ch_p, "biasr": bias_p}
        for i in range(N_CORES)
    ]

    nc = _get_nc()
    _LAST_IN_MAPS = in_maps
    br = run_bass_kernel_spmd(nc, in_maps, list(range(N_CORES)))
    LAST_BR = br

    idx_all = np.empty((N_CORES, 128, NT), dtype=np.int64)
    repair = []                                       # (core, p, t) triples
    for i in range(N_CORES):
        acc = br.results[i]["oacc"].astype(np.int64)  # (128, NT, 2) j_h
        mm = br.results[i]["omax"]                    # (128, NT, 2) m_h
        hstar = (mm[:, :, 1] > mm[:, :, 0]).astype(np.int64)
        j_h = np.where(hstar == 1, acc[:, :, 1], acc[:, :, 0])
        idx_all[i] = hstar * KH + j_h
        bad = np.nonzero((acc[:, :, 0] == KH - 1) | (acc[:, :, 1] == KH - 1))
        repair.extend((i, int(p), int(t)) for p, t in zip(*bad))

    if repair:
        # argmax at a half's last slot -> that half's max value is unknown
        # (the running-max output was overwritten by the hit index).
        # Recompute those few points exactly (same fp16-quantized math).
        pts = np.array([core * PTS_PER_CORE + t * 128 + p
                        for core, p, t in repair], dtype=np.int64)
        sc = (v16[pts].astype(np.float32) @ c16.T.astype(np.float32)
              + bias[None, :])
        fixed = np.argmax(sc, axis=1)
        for (core, p, t), f in zip(repair, fixed):
            idx_all[core, p, t] = int(f)

    parts = [idx_all[i].T.reshape(-1) for i in range(N_CORES)]
    return np.concatenate(parts).astype(np.int32)


_LAST_IN_MAPS = None


def _install_ntff_shim():
    """antenv.axon_hooks is missing in some images; rebuild it from the boot
    helper so run_bass_kernel_spmd(trace=True) can profile via NTFF."""
    import sys, types
    try:
        from antenv.axon_hooks import get_axon_ntff_profile_hook  # noqa: F401
        return True
    except ImportError:
        pass
    try:
        from trn_agent_boot.trn_boot import _ntff_profile_via_ctypes
        hook = _ntff_profile_via_ctypes('/opt/axon/libaxon_pjrt.so')
        mod = types.ModuleType("antenv.axon_hooks")
        mod.get_axon_ntff_profile_hook = lambda: hook
        mod.set_axon_ntff_profile_hook = lambda h: None
        sys.modules["antenv.axon_hooks"] = mod
        return True
    except Exception:
        return False


def measure_exec_ns(reps: int = 3) -> int:
    """Real HW execution time from a neuron-profile (NTFF) capture; falls
    back to best-of-N wall clock around the execute if profiling is
    unavailable."""
    import tempfile
    import time
    nc = _get_nc()
    assert _LAST_IN_MAPS is not None, "call kernel() first"
    try:
        _install_ntff_shim()
        tmpdir = tempfile.mkdtemp(prefix="kmeans_ntff_")
        br = run_bass_kernel_spmd(nc, _LAST_IN_MAPS, list(range(N_CORES)),
                                  trace=True, tmpdir=tmpdir)
        if br.exec_time_ns is not None:
            return int(br.exec_time_ns)
    except Exception:
        pass
    best = None
    for _ in range(max(1, reps)):
        t0 = time.perf_counter()
        run_bass_kernel_spmd(nc, _LAST_IN_MAPS, list(range(N_CORES)))
        dt = time.perf_counter() - t0
        best = dt if best is None else min(best, dt)
    return int(best * 1e9)
